# revision 8
# baseline (speedup 1.0000x reference)
"""NgramHasher Trainium2 kernel (v2: 4x row-tiled matmuls).

Computes h[b,s,ch] = (sum_j coeffs[k,j] * window_j) mod 2^20 for ngram sizes
(2, 3) x 8 tables, on 8 NeuronCores (data parallel over batch).

Math: with c = c0 + 2^10*c1 and t = t0 + 2^10*t1 (10-bit chunks),
  h = (A + 2^10 * (B mod 2^10)) mod 2^20
  A = sum_j c0[j]*t0[s-j]                      (fp16 matmul, exact in fp32)
  B = sum_j (c0[j]*t1[s-j] + c1[j]*t0[s-j])    (fp16 matmul)
B rides a +2^23 bias so its fp32 mantissa IS B; one DVE tensor_scalar turns
(bits & 0x3FF) into an fp16 with value 1024 + (B mod 1024); a scaled-identity
matmul (1024*I) accumulates 2^10*that into A's PSUM bank. Host masks 0xFFFFF.

v2 performance structure (PE is pinned at 1.2 GHz here - no HAM warmup):
  - The WB/WA matmuls have K=21 <= 32, so they run as 4 CONCURRENT 32-row
    tiles via tile_position=(32*i, 0). X is packed [128, 2048]: partition
    32*i+j holds X row j for global column block i (block = 2048 cols).
    Each "sigma" step does 4 banks (4 blocks x 512 cols) in ~512 PE cycles
    per pass instead of 2048.
  - The WI identity matmul is inherently K=128 (per-partition scale-add),
    one 512-col matmul per bank.
  - Per-bank pipelining: PSUM pool of 8 [128,512] tiles (8 banks); B and A
    share a bank (WA start=True overwrites after the DVE u-extract reads B).
  - Tensor issue order is software-pipelined (WB of sigma+1 issued before
    WA/WI of sigma) so the PE has work during the DVE extract latency.
"""
import sys
sys.path.insert(0, "/opt/trn_rl_repo")
import numpy as np
from contextlib import ExitStack
from numpy.lib.stride_tricks import sliding_window_view

import concourse.bass as bass
import concourse.tile as tile
from concourse import bacc, mybir
from concourse.bass_utils import run_bass_kernel_spmd

dt = mybir.dt
AluOp = mybir.AluOpType

N_CORES = 8
B, S = 64, 8192
B_LOC = B // N_CORES            # batch rows per core
P_CORE = B_LOC * S              # positions per core (65536)
G = 8                           # position offsets packed into M
NCH = 16                        # output channels (2 ngram sizes x 8 tables)
Q = P_CORE // G                 # moving columns per core (8192)
NTILE = 4                       # concurrent row-tile groups
BLK = Q // NTILE                # global columns per row-tile block (2048)
BANK = 512                      # fp32 columns per PSUM bank
NSIG = BLK // BANK              # sigma steps (4)

_NC_CACHE = {}


def _build_bass():
    """Build the SPMD Bass program (identical on all 8 cores)."""
    nc = bacc.Bacc("TRN2", target_bir_lowering=False, debug=False,
                   num_devices=N_CORES)
    x_d = nc.dram_tensor("X", [128, BLK], dt.float16, kind="ExternalInput").ap()
    # W = [WB | WA | WI] concatenated along the free dim: one input DMA
    w_d = nc.dram_tensor("W", [128, 384], dt.float16, kind="ExternalInput").ap()
    out_d = nc.dram_tensor("OUT", [128, NTILE, BLK], dt.uint32,
                           kind="ExternalOutput").ap()

    with tile.TileContext(nc) as tc:
        with ExitStack() as ctx:
            wpool = ctx.enter_context(tc.tile_pool(name="w", bufs=1))
            xpool = ctx.enter_context(tc.tile_pool(name="x", bufs=NSIG + 1))
            upool = ctx.enter_context(
                tc.tile_pool(name="u", bufs=NSIG * NTILE + 1))
            opool = ctx.enter_context(
                tc.tile_pool(name="o", bufs=NSIG * NTILE // 2 + 1))
            # separate B and A PSUM pools (4 banks each): WA has no
            # dependency on the DVE u-extract, so WA groups launch
            # concurrently right after WB
            psb = ctx.enter_context(
                tc.tile_pool(name="psb", bufs=NTILE, space="PSUM"))
            psa = ctx.enter_context(
                tc.tile_pool(name="psa", bufs=NTILE, space="PSUM"))

            w_t = wpool.tile([128, 384], dt.float16, tag="w")
            nc.sync.dma_start(w_t[:], w_d[:])
            w_b = w_t[:, 0:128]
            w_a = w_t[:, 128:256]
            w_i = w_t[:, 256:384]

            xts = []
            for s in range(NSIG):
                xt = xpool.tile([128, BANK], dt.float16, tag="xt")
                # scalar-queue HWDGE ring: parallel to the sync ring
                nc.scalar.dma_start(xt[:], x_d[:, s * BANK:(s + 1) * BANK])
                xts.append(xt)

            for s in range(NSIG):
                pb = [psb.tile([128, BANK], dt.float32, tag="psb",
                               name=f"psb_{s}_{i}") for i in range(NTILE)]
                pa = [psa.tile([128, BANK], dt.float32, tag="psa",
                               name=f"psa_{s}_{i}") for i in range(NTILE)]
                us = [upool.tile([128, BANK], dt.uint16, tag="u",
                                 name=f"u_{s}_{i}") for i in range(NTILE)]
                for i in range(NTILE):
                    nc.tensor.matmul(
                        pb[i][:], w_b[32 * i:32 * i + 21, :],
                        xts[s][32 * i:32 * i + 21, :],
                        start=True, stop=True, tile_position=(32 * i, 0))
                for i in range(NTILE):
                    nc.tensor.matmul(
                        pa[i][:], w_a[32 * i:32 * i + 21, :],
                        xts[s][32 * i:32 * i + 21, :],
                        start=True, stop=False, tile_position=(32 * i, 0),
                        skip_group_check=True)
                for i in range(NTILE):
                    nc.vector.tensor_scalar(
                        us[i][:], pb[i][:].bitcast(dt.uint16)[:, ::2],
                        0x3FF, 25 << 10,
                        AluOp.bitwise_and, AluOp.bitwise_or)
                for i in range(NTILE):
                    nc.tensor.matmul(
                        pa[i][:], w_i[:],
                        us[i][:].bitcast(dt.float16),
                        start=False, stop=True, skip_group_check=True)
                for half in range(2):
                    o = opool.tile([128, 2 * BANK], dt.uint32, tag="o",
                                   name=f"o_{s}_{half}")
                    for k in range(2):
                        nc.scalar.copy(o[:, k * BANK:(k + 1) * BANK],
                                       pa[2 * half + k][:])
                    nc.sync.dma_start(
                        out_d[:, 2 * half:2 * half + 2,
                              s * BANK:(s + 1) * BANK],
                        o[:].rearrange("p (i c) -> p i c", i=2))
    nc.compile()
    return nc


def _get_nc():
    if "nc" not in _NC_CACHE:
        _NC_CACHE["nc"] = _build_bass()
    return _NC_CACHE["nc"]


def _band(cpart):
    """[8,3] coeff chunk -> banded Toeplitz [10, 128] weight (fp32 values)."""
    W = np.zeros((10, 128), np.float32)
    for g in range(G):
        for k in range(8):
            for j in range(2):              # ngram n=2 -> channels 0..7
                W[g + 1 + j, g * 16 + k] = cpart[k, j]
            for j in range(3):              # ngram n=3 -> channels 8..15
                W[g + j, g * 16 + 8 + k] = cpart[k, j]
    return W


def _pack_rowgroups(w21):
    """[21, 128] weight -> [128, 128] with 4 copies at partition 32*i."""
    W = np.zeros((128, 128), np.float16)
    for i in range(NTILE):
        W[32 * i:32 * i + 21, :] = w21
    return W


def _host_prep(token_ids, coeffs):
    t = np.asarray(token_ids).astype(np.int64)
    c = np.asarray(coeffs).astype(np.int64)

    t0 = (t & 0x3FF).astype(np.float16)     # [64, 8192]
    t1 = (t >> 10).astype(np.float16)
    pad = np.zeros((B, 2), np.float16)
    t0p = np.concatenate([pad, t0], axis=1)  # [64, 8194]
    t1p = np.concatenate([pad, t1], axis=1)
    # w?[b, q_loc, r] = t?p[b, 8*q_loc + r],  q_loc in [0,1024), r in [0,10)
    w0 = sliding_window_view(t0p, 10, axis=1)[:, ::G, :]
    w1 = sliding_window_view(t1p, 10, axis=1)[:, ::G, :]
    w0 = np.ascontiguousarray(w0.transpose(0, 2, 1))  # [64, 10, 1024]
    w1 = np.ascontiguousarray(w1.transpose(0, 2, 1))

    c0 = (c & 0x3FF).astype(np.float32)
    c1 = (c >> 10).astype(np.float32)
    # 2^23 bias arrives as (2^15 weight) * (2^8 const input row): fp16-exact
    bias_row = np.full((1, 128), float(1 << 15), np.float32)
    WB = np.concatenate([_band(c0), _band(c1), bias_row],
                        axis=0).astype(np.float16)
    WA = np.concatenate([np.zeros((10, 128), np.float32), _band(c0), bias_row],
                        axis=0).astype(np.float16)
    WBP = _pack_rowgroups(WB)
    WAP = _pack_rowgroups(WA)
    WIP = (1024.0 * np.eye(128)).astype(np.float16)
    WCAT = np.concatenate([WBP, WAP, WIP], axis=1)  # [128, 384]

    in_maps = []
    for core in range(N_CORES):
        b0 = core * B_LOC
        X = np.empty((21, Q), np.float16)
        # rows 0..9: X1 windows; rows 10..19: X0 windows; row 20: const
        X[0:10] = w1[b0:b0 + B_LOC].transpose(1, 0, 2).reshape(10, Q)
        X[10:20] = w0[b0:b0 + B_LOC].transpose(1, 0, 2).reshape(10, Q)
        X[20] = 256.0
        # pack into row groups: partition 32*i+j = X[j, i*BLK:(i+1)*BLK]
        XP = np.zeros((128, BLK), np.float16)
        for i in range(NTILE):
            XP[32 * i:32 * i + 21, :] = X[:, i * BLK:(i + 1) * BLK]
        in_maps.append({"X": XP, "W": WCAT})
    return in_maps


def _unshard(results):
    out = np.empty((B, S, NCH), np.int64)
    for core, res in enumerate(results):
        o = (res["OUT"].reshape(128, Q) & 0xFFFFF).reshape(G, NCH, Q)
        o = o.transpose(2, 0, 1).reshape(P_CORE, NCH)  # [8q+g, ch]
        out[core * B_LOC:(core + 1) * B_LOC] = \
            o.reshape(B_LOC, S, NCH).astype(np.int64)
    return out


def _run(token_ids, coeffs, **spmd_kwargs):
    in_maps = _host_prep(token_ids, coeffs)
    nc = _get_nc()
    res = run_bass_kernel_spmd(nc, in_maps, core_ids=list(range(N_CORES)),
                               **spmd_kwargs)
    return _unshard(res.results), res


def kernel(token_ids, coeffs):
    out, _ = _run(token_ids, coeffs)
    return out


# revision 12
# speedup vs baseline: 1.0753x; 1.0753x over previous
"""NgramHasher Trainium2 kernel (v2: 4x row-tiled matmuls).

Computes h[b,s,ch] = (sum_j coeffs[k,j] * window_j) mod 2^20 for ngram sizes
(2, 3) x 8 tables, on 8 NeuronCores (data parallel over batch).

Math: with c = c0 + 2^10*c1 and t = t0 + 2^10*t1 (10-bit chunks),
  h = (A + 2^10 * (B mod 2^10)) mod 2^20
  A = sum_j c0[j]*t0[s-j]                      (fp16 matmul, exact in fp32)
  B = sum_j (c0[j]*t1[s-j] + c1[j]*t0[s-j])    (fp16 matmul)
B rides a +2^23 bias so its fp32 mantissa IS B; one DVE tensor_scalar turns
(bits & 0x3FF) into an fp16 with value 1024 + (B mod 1024); a scaled-identity
matmul (1024*I) accumulates 2^10*that into A's PSUM bank. Host masks 0xFFFFF.

v2 performance structure (PE is pinned at 1.2 GHz here - no HAM warmup):
  - The WB/WA matmuls have K=21 <= 32, so they run as 4 CONCURRENT 32-row
    tiles via tile_position=(32*i, 0). X is packed [128, 2048]: partition
    32*i+j holds X row j for global column block i (block = 2048 cols).
    Each "sigma" step does 4 banks (4 blocks x 512 cols) in ~512 PE cycles
    per pass instead of 2048.
  - The WI identity matmul is inherently K=128 (per-partition scale-add),
    one 512-col matmul per bank.
  - Per-bank pipelining: PSUM pool of 8 [128,512] tiles (8 banks); B and A
    share a bank (WA start=True overwrites after the DVE u-extract reads B).
  - Tensor issue order is software-pipelined (WB of sigma+1 issued before
    WA/WI of sigma) so the PE has work during the DVE extract latency.
"""
import sys
sys.path.insert(0, "/opt/trn_rl_repo")
import numpy as np
from contextlib import ExitStack
from numpy.lib.stride_tricks import sliding_window_view

import concourse.bass as bass
import concourse.tile as tile
from concourse import bacc, mybir
from concourse.bass_utils import run_bass_kernel_spmd

dt = mybir.dt
AluOp = mybir.AluOpType

N_CORES = 8
B, S = 64, 8192
B_LOC = B // N_CORES            # batch rows per core
P_CORE = B_LOC * S              # positions per core (65536)
G = 8                           # position offsets packed into M
NCH = 16                        # output channels (2 ngram sizes x 8 tables)
Q = P_CORE // G                 # moving columns per core (8192)
NTILE = 4                       # concurrent row-tile groups
BLK = Q // NTILE                # global columns per row-tile block (2048)
BANK = 512                      # fp32 columns per PSUM bank
NSIG = BLK // BANK              # sigma steps (4)

_NC_CACHE = {}


def _build_bass():
    """Build the SPMD Bass program (identical on all 8 cores)."""
    nc = bacc.Bacc("TRN2", target_bir_lowering=False, debug=False,
                   num_devices=N_CORES)
    x_d = nc.dram_tensor("X", [128, BLK], dt.float16, kind="ExternalInput").ap()
    # W = [WB | WA | WI] concatenated along the free dim: one input DMA
    w_d = nc.dram_tensor("W", [128, 384], dt.float16, kind="ExternalInput").ap()
    out_d = nc.dram_tensor("OUT", [128, NTILE, BLK], dt.uint32,
                           kind="ExternalOutput").ap()

    with tile.TileContext(nc) as tc:
        with ExitStack() as ctx:
            wpool = ctx.enter_context(tc.tile_pool(name="w", bufs=1))
            xpool = ctx.enter_context(tc.tile_pool(name="x", bufs=NSIG + 1))
            upool = ctx.enter_context(
                tc.tile_pool(name="u", bufs=NSIG * NTILE + 1))
            opool = ctx.enter_context(
                tc.tile_pool(name="o", bufs=NSIG * NTILE + 1))
            # separate B and A PSUM pools (4 banks each): WA has no
            # dependency on the DVE u-extract, so WA groups launch
            # concurrently right after WB
            psb = ctx.enter_context(
                tc.tile_pool(name="psb", bufs=NTILE, space="PSUM"))
            psa = ctx.enter_context(
                tc.tile_pool(name="psa", bufs=NTILE, space="PSUM"))

            w_t = wpool.tile([128, 384], dt.float16, tag="w")
            nc.sync.dma_start(w_t[:], w_d[:])
            w_b = w_t[:, 0:128]
            w_a = w_t[:, 128:256]
            w_i = w_t[:, 256:384]

            xts = []
            for s in range(NSIG):
                xt = xpool.tile([128, BANK], dt.float16, tag="xt")
                # scalar-queue HWDGE ring: parallel to the sync ring
                nc.scalar.dma_start(xt[:], x_d[:, s * BANK:(s + 1) * BANK])
                xts.append(xt)

            # per-sigma state, for the skewed pipeline below
            PB = [None] * NSIG     # 4x [128,512] B-plane psum tiles
            PA = [None] * NSIG     # 2x [128,1024] A-plane psum tiles (pairs)
            US = [None] * NSIG     # 4x [128,512] u16 extracted tiles

            def emit_wb(s):
                PB[s] = [psb.tile([128, BANK], dt.float32, tag="psb",
                                  name=f"psb_{s}_{i}") for i in range(NTILE)]
                for i in range(NTILE):
                    nc.tensor.matmul(
                        PB[s][i][:], w_b[32 * i:32 * i + 21, :],
                        xts[s][32 * i:32 * i + 21, :],
                        start=True, stop=True, tile_position=(32 * i, 0))

            def emit_wa(s):
                PA[s] = [psa.tile([128, BANK], dt.float32, tag="psa",
                                  name=f"psa_{s}_{i}") for i in range(NTILE)]
                for i in range(NTILE):
                    nc.tensor.matmul(
                        PA[s][i][:],
                        w_a[32 * i:32 * i + 21, :],
                        xts[s][32 * i:32 * i + 21, :],
                        start=True, stop=False, tile_position=(32 * i, 0),
                        skip_group_check=True)

            def emit_u(s):
                US[s] = [upool.tile([128, BANK], dt.uint16, tag="u",
                                    name=f"u_{s}_{i}") for i in range(NTILE)]
                for i in range(NTILE):
                    nc.vector.tensor_scalar(
                        US[s][i][:], PB[s][i][:].bitcast(dt.uint16)[:, ::2],
                        0x3FF, 25 << 10,
                        AluOp.bitwise_and, AluOp.bitwise_or)

            def emit_wi(s):
                for i in range(NTILE):
                    nc.tensor.matmul(
                        PA[s][i][:],
                        w_i[:],
                        US[s][i][:].bitcast(dt.float16),
                        start=False, stop=True, skip_group_check=True)

            def emit_out(s):
                for i in range(NTILE):
                    o = opool.tile([128, BANK], dt.uint32, tag="o",
                                   name=f"o_{s}_{i}")
                    nc.scalar.copy(o[:], PA[s][i][:])
                    nc.sync.dma_start(
                        out_d[:, i, s * BANK:(s + 1) * BANK], o[:])

            # stage-major skewed emission: per-engine program order matches
            # the intended execution order, so nothing queue-blocks
            emit_wb(0)
            emit_wa(0)
            emit_u(0)
            for s in range(1, NSIG):
                emit_wb(s)
                emit_wi(s - 1)
                emit_out(s - 1)
                emit_wa(s)
                emit_u(s)
            emit_wi(NSIG - 1)
            emit_out(NSIG - 1)
    nc.compile()
    return nc


def _get_nc():
    if "nc" not in _NC_CACHE:
        _NC_CACHE["nc"] = _build_bass()
    return _NC_CACHE["nc"]


def _band(cpart):
    """[8,3] coeff chunk -> banded Toeplitz [10, 128] weight (fp32 values)."""
    W = np.zeros((10, 128), np.float32)
    for g in range(G):
        for k in range(8):
            for j in range(2):              # ngram n=2 -> channels 0..7
                W[g + 1 + j, g * 16 + k] = cpart[k, j]
            for j in range(3):              # ngram n=3 -> channels 8..15
                W[g + j, g * 16 + 8 + k] = cpart[k, j]
    return W


def _pack_rowgroups(w21):
    """[21, 128] weight -> [128, 128] with 4 copies at partition 32*i."""
    W = np.zeros((128, 128), np.float16)
    for i in range(NTILE):
        W[32 * i:32 * i + 21, :] = w21
    return W


def _host_prep(token_ids, coeffs):
    t = np.asarray(token_ids).astype(np.int64)
    c = np.asarray(coeffs).astype(np.int64)

    t0 = (t & 0x3FF).astype(np.float16)     # [64, 8192]
    t1 = (t >> 10).astype(np.float16)
    pad = np.zeros((B, 2), np.float16)
    t0p = np.concatenate([pad, t0], axis=1)  # [64, 8194]
    t1p = np.concatenate([pad, t1], axis=1)
    # w?[b, q_loc, r] = t?p[b, 8*q_loc + r],  q_loc in [0,1024), r in [0,10)
    w0 = sliding_window_view(t0p, 10, axis=1)[:, ::G, :]
    w1 = sliding_window_view(t1p, 10, axis=1)[:, ::G, :]
    w0 = np.ascontiguousarray(w0.transpose(0, 2, 1))  # [64, 10, 1024]
    w1 = np.ascontiguousarray(w1.transpose(0, 2, 1))

    c0 = (c & 0x3FF).astype(np.float32)
    c1 = (c >> 10).astype(np.float32)
    # 2^23 bias arrives as (2^15 weight) * (2^8 const input row): fp16-exact
    bias_row = np.full((1, 128), float(1 << 15), np.float32)
    WB = np.concatenate([_band(c0), _band(c1), bias_row],
                        axis=0).astype(np.float16)
    WA = np.concatenate([np.zeros((10, 128), np.float32), _band(c0), bias_row],
                        axis=0).astype(np.float16)
    WBP = _pack_rowgroups(WB)
    WAP = _pack_rowgroups(WA)
    WIP = (1024.0 * np.eye(128)).astype(np.float16)
    WCAT = np.concatenate([WBP, WAP, WIP], axis=1)  # [128, 384]

    in_maps = []
    for core in range(N_CORES):
        b0 = core * B_LOC
        X = np.empty((21, Q), np.float16)
        # rows 0..9: X1 windows; rows 10..19: X0 windows; row 20: const
        X[0:10] = w1[b0:b0 + B_LOC].transpose(1, 0, 2).reshape(10, Q)
        X[10:20] = w0[b0:b0 + B_LOC].transpose(1, 0, 2).reshape(10, Q)
        X[20] = 256.0
        # pack into row groups: partition 32*i+j = X[j, i*BLK:(i+1)*BLK]
        XP = np.zeros((128, BLK), np.float16)
        for i in range(NTILE):
            XP[32 * i:32 * i + 21, :] = X[:, i * BLK:(i + 1) * BLK]
        in_maps.append({"X": XP, "W": WCAT})
    return in_maps


def _unshard(results):
    out = np.empty((B, S, NCH), np.int64)
    for core, res in enumerate(results):
        o = (res["OUT"].reshape(128, Q) & 0xFFFFF).reshape(G, NCH, Q)
        o = o.transpose(2, 0, 1).reshape(P_CORE, NCH)  # [8q+g, ch]
        out[core * B_LOC:(core + 1) * B_LOC] = \
            o.reshape(B_LOC, S, NCH).astype(np.int64)
    return out


def _run(token_ids, coeffs, **spmd_kwargs):
    in_maps = _host_prep(token_ids, coeffs)
    nc = _get_nc()
    res = run_bass_kernel_spmd(nc, in_maps, core_ids=list(range(N_CORES)),
                               **spmd_kwargs)
    return _unshard(res.results), res


def kernel(token_ids, coeffs):
    out, _ = _run(token_ids, coeffs)
    return out


# revision 14
# speedup vs baseline: 1.2040x; 1.1197x over previous
"""NgramHasher Trainium2 kernel (v2: 4x row-tiled matmuls).

Computes h[b,s,ch] = (sum_j coeffs[k,j] * window_j) mod 2^20 for ngram sizes
(2, 3) x 8 tables, on 8 NeuronCores (data parallel over batch).

Math: with c = c0 + 2^10*c1 and t = t0 + 2^10*t1 (10-bit chunks),
  h = (A + 2^10 * (B mod 2^10)) mod 2^20
  A = sum_j c0[j]*t0[s-j]                      (fp16 matmul, exact in fp32)
  B = sum_j (c0[j]*t1[s-j] + c1[j]*t0[s-j])    (fp16 matmul)
B rides a +2^23 bias so its fp32 mantissa IS B; one DVE tensor_scalar turns
(bits & 0x3FF) into an fp16 with value 1024 + (B mod 1024); a scaled-identity
matmul (1024*I) accumulates 2^10*that into A's PSUM bank. Host masks 0xFFFFF.

v2 performance structure (PE is pinned at 1.2 GHz here - no HAM warmup):
  - The WB/WA matmuls have K=21 <= 32, so they run as 4 CONCURRENT 32-row
    tiles via tile_position=(32*i, 0). X is packed [128, 2048]: partition
    32*i+j holds X row j for global column block i (block = 2048 cols).
    Each "sigma" step does 4 banks (4 blocks x 512 cols) in ~512 PE cycles
    per pass instead of 2048.
  - The WI identity matmul is inherently K=128 (per-partition scale-add),
    one 512-col matmul per bank.
  - Per-bank pipelining: PSUM pool of 8 [128,512] tiles (8 banks); B and A
    share a bank (WA start=True overwrites after the DVE u-extract reads B).
  - Tensor issue order is software-pipelined (WB of sigma+1 issued before
    WA/WI of sigma) so the PE has work during the DVE extract latency.
"""
import sys
sys.path.insert(0, "/opt/trn_rl_repo")
import numpy as np
from contextlib import ExitStack
from numpy.lib.stride_tricks import sliding_window_view

import concourse.bass as bass
import concourse.tile as tile
from concourse import bacc, mybir
from concourse.bass_utils import run_bass_kernel_spmd

dt = mybir.dt
AluOp = mybir.AluOpType

N_CORES = 8
B, S = 64, 8192
B_LOC = B // N_CORES            # batch rows per core
P_CORE = B_LOC * S              # positions per core (65536)
G = 8                           # position offsets packed into M
NCH = 16                        # output channels (2 ngram sizes x 8 tables)
Q = P_CORE // G                 # moving columns per core (8192)
NTILE = 4                       # concurrent row-tile groups
BLK = Q // NTILE                # global columns per row-tile block (2048)
BANK = 512                      # fp32 columns per PSUM bank
NSIG = BLK // BANK              # sigma steps (4)

_NC_CACHE = {}


def _build_bass():
    """Build the SPMD Bass program (identical on all 8 cores)."""
    nc = bacc.Bacc("TRN2", target_bir_lowering=False, debug=False,
                   num_devices=N_CORES)
    x_d = nc.dram_tensor("X", [128, BLK], dt.float16, kind="ExternalInput").ap()
    # W = [WB | WA | WI] concatenated along the free dim: one input DMA
    w_d = nc.dram_tensor("W", [128, 384], dt.float16, kind="ExternalInput").ap()
    out_d = nc.dram_tensor("OUT", [128, NTILE, BLK], dt.uint32,
                           kind="ExternalOutput").ap()

    with tile.TileContext(nc) as tc:
        with ExitStack() as ctx:
            wpool = ctx.enter_context(tc.tile_pool(name="w", bufs=1))
            xpool = ctx.enter_context(tc.tile_pool(name="x", bufs=NSIG + 1))
            upool = ctx.enter_context(
                tc.tile_pool(name="u", bufs=NSIG * NTILE + 1))
            opool = ctx.enter_context(
                tc.tile_pool(name="o", bufs=NSIG * NTILE + 1))
            # separate B and A PSUM pools (4 banks each, as 2x 2-bank pair
            # tiles): WA has no dependency on the DVE u-extract, so WA
            # groups launch concurrently right after WB; pair-granular
            # DVE/ACT ops keep the scheduler from threading per-bank
            psb = ctx.enter_context(
                tc.tile_pool(name="psb", bufs=2, space="PSUM"))
            psa = ctx.enter_context(
                tc.tile_pool(name="psa", bufs=2, space="PSUM"))

            w_t = wpool.tile([128, 384], dt.float16, tag="w")
            nc.sync.dma_start(w_t[:], w_d[:])
            w_b = w_t[:, 0:128]
            w_a = w_t[:, 128:256]
            w_i = w_t[:, 256:384]

            xts = []
            for s in range(NSIG):
                xt = xpool.tile([128, BANK], dt.float16, tag="xt")
                # scalar-queue HWDGE ring: parallel to the sync ring
                nc.scalar.dma_start(xt[:], x_d[:, s * BANK:(s + 1) * BANK])
                xts.append(xt)

            # per-sigma state, for the skewed pipeline below
            PB = [None] * NSIG     # 4x [128,512] B-plane psum tiles
            PA = [None] * NSIG     # 2x [128,1024] A-plane psum tiles (pairs)
            US = [None] * NSIG     # 4x [128,512] u16 extracted tiles

            def emit_wb(s):
                PB[s] = [psb.tile([128, 2 * BANK], dt.float32, tag="psb",
                                  name=f"psb_{s}_{p}") for p in range(2)]
                for i in range(NTILE):
                    nc.tensor.matmul(
                        PB[s][i // 2][:, (i % 2) * BANK:(i % 2 + 1) * BANK],
                        w_b[32 * i:32 * i + 21, :],
                        xts[s][32 * i:32 * i + 21, :],
                        start=True, stop=True, tile_position=(32 * i, 0))

            def emit_wa(s):
                PA[s] = [psa.tile([128, 2 * BANK], dt.float32, tag="psa",
                                  name=f"psa_{s}_{p}") for p in range(2)]
                for i in range(NTILE):
                    nc.tensor.matmul(
                        PA[s][i // 2][:, (i % 2) * BANK:(i % 2 + 1) * BANK],
                        w_a[32 * i:32 * i + 21, :],
                        xts[s][32 * i:32 * i + 21, :],
                        start=True, stop=False, tile_position=(32 * i, 0),
                        skip_group_check=True)

            def emit_u(s):
                US[s] = [upool.tile([128, 2 * BANK], dt.uint16, tag="u",
                                    name=f"u_{s}_{p}") for p in range(2)]
                for p in range(2):
                    nc.vector.tensor_scalar(
                        US[s][p][:], PB[s][p][:].bitcast(dt.uint16)[:, ::2],
                        0x3FF, 25 << 10,
                        AluOp.bitwise_and, AluOp.bitwise_or)

            def emit_wi(s):
                for i in range(NTILE):
                    nc.tensor.matmul(
                        PA[s][i // 2][:, (i % 2) * BANK:(i % 2 + 1) * BANK],
                        w_i[:],
                        US[s][i // 2][:, (i % 2) * BANK:(i % 2 + 1) * BANK]
                        .bitcast(dt.float16),
                        start=False, stop=True, skip_group_check=True)

            def emit_out(s):
                for p in range(2):
                    o = opool.tile([128, 2 * BANK], dt.uint32, tag="o",
                                   name=f"o_{s}_{p}")
                    nc.scalar.copy(o[:], PA[s][p][:])
                    nc.sync.dma_start(
                        out_d[:, 2 * p:2 * p + 2, s * BANK:(s + 1) * BANK],
                        o[:].rearrange("p (i c) -> p i c", i=2))

            # stage-major skewed emission: per-engine program order matches
            # the intended execution order, so nothing queue-blocks
            emit_wb(0)
            emit_wa(0)
            emit_u(0)
            for s in range(1, NSIG):
                emit_wb(s)
                emit_wi(s - 1)
                emit_out(s - 1)
                emit_wa(s)
                emit_u(s)
            emit_wi(NSIG - 1)
            emit_out(NSIG - 1)
    nc.compile()
    return nc


def _get_nc():
    if "nc" not in _NC_CACHE:
        _NC_CACHE["nc"] = _build_bass()
    return _NC_CACHE["nc"]


def _band(cpart):
    """[8,3] coeff chunk -> banded Toeplitz [10, 128] weight (fp32 values)."""
    W = np.zeros((10, 128), np.float32)
    for g in range(G):
        for k in range(8):
            for j in range(2):              # ngram n=2 -> channels 0..7
                W[g + 1 + j, g * 16 + k] = cpart[k, j]
            for j in range(3):              # ngram n=3 -> channels 8..15
                W[g + j, g * 16 + 8 + k] = cpart[k, j]
    return W


def _pack_rowgroups(w21):
    """[21, 128] weight -> [128, 128] with 4 copies at partition 32*i."""
    W = np.zeros((128, 128), np.float16)
    for i in range(NTILE):
        W[32 * i:32 * i + 21, :] = w21
    return W


def _host_prep(token_ids, coeffs):
    t = np.asarray(token_ids).astype(np.int64)
    c = np.asarray(coeffs).astype(np.int64)

    t0 = (t & 0x3FF).astype(np.float16)     # [64, 8192]
    t1 = (t >> 10).astype(np.float16)
    pad = np.zeros((B, 2), np.float16)
    t0p = np.concatenate([pad, t0], axis=1)  # [64, 8194]
    t1p = np.concatenate([pad, t1], axis=1)
    # w?[b, q_loc, r] = t?p[b, 8*q_loc + r],  q_loc in [0,1024), r in [0,10)
    w0 = sliding_window_view(t0p, 10, axis=1)[:, ::G, :]
    w1 = sliding_window_view(t1p, 10, axis=1)[:, ::G, :]
    w0 = np.ascontiguousarray(w0.transpose(0, 2, 1))  # [64, 10, 1024]
    w1 = np.ascontiguousarray(w1.transpose(0, 2, 1))

    c0 = (c & 0x3FF).astype(np.float32)
    c1 = (c >> 10).astype(np.float32)
    # 2^23 bias arrives as (2^15 weight) * (2^8 const input row): fp16-exact
    bias_row = np.full((1, 128), float(1 << 15), np.float32)
    WB = np.concatenate([_band(c0), _band(c1), bias_row],
                        axis=0).astype(np.float16)
    WA = np.concatenate([np.zeros((10, 128), np.float32), _band(c0), bias_row],
                        axis=0).astype(np.float16)
    WBP = _pack_rowgroups(WB)
    WAP = _pack_rowgroups(WA)
    WIP = (1024.0 * np.eye(128)).astype(np.float16)
    WCAT = np.concatenate([WBP, WAP, WIP], axis=1)  # [128, 384]

    in_maps = []
    for core in range(N_CORES):
        b0 = core * B_LOC
        X = np.empty((21, Q), np.float16)
        # rows 0..9: X1 windows; rows 10..19: X0 windows; row 20: const
        X[0:10] = w1[b0:b0 + B_LOC].transpose(1, 0, 2).reshape(10, Q)
        X[10:20] = w0[b0:b0 + B_LOC].transpose(1, 0, 2).reshape(10, Q)
        X[20] = 256.0
        # pack into row groups: partition 32*i+j = X[j, i*BLK:(i+1)*BLK]
        XP = np.zeros((128, BLK), np.float16)
        for i in range(NTILE):
            XP[32 * i:32 * i + 21, :] = X[:, i * BLK:(i + 1) * BLK]
        in_maps.append({"X": XP, "W": WCAT})
    return in_maps


def _unshard(results):
    out = np.empty((B, S, NCH), np.int64)
    for core, res in enumerate(results):
        o = (res["OUT"].reshape(128, Q) & 0xFFFFF).reshape(G, NCH, Q)
        o = o.transpose(2, 0, 1).reshape(P_CORE, NCH)  # [8q+g, ch]
        out[core * B_LOC:(core + 1) * B_LOC] = \
            o.reshape(B_LOC, S, NCH).astype(np.int64)
    return out


def _run(token_ids, coeffs, **spmd_kwargs):
    in_maps = _host_prep(token_ids, coeffs)
    nc = _get_nc()
    res = run_bass_kernel_spmd(nc, in_maps, core_ids=list(range(N_CORES)),
                               **spmd_kwargs)
    return _unshard(res.results), res


def kernel(token_ids, coeffs):
    out, _ = _run(token_ids, coeffs)
    return out
